# revision 34
# baseline (speedup 1.0000x reference)
"""Trainium2 Bass kernel for nn_ConsistencyLoss (BCE + dilated-stencil consistency loss).

loss = mean( unfolded_weights * thred + bce )
  bce      = -(y_true*max(log(y_pred),-100) + (1-y_true)*max(log1p(-y_pred),-100))
  unfolded = max over 8 dilated (DIL=2) neighbors nb of |y_pred - nb|, zero-padded
  thred    = y_pred * (y_pred >= 0.5)

Strategy (8 NeuronCores, data-parallel over batch, 2 images/core):
  - Chunk tiles [128, 4096] = 2 bands x 2 images, blocks [i0b0|i0b1|i1b0|i1b1].
  - unfolded = max(c - nmin, nmax - c); nmax/nmin separable over the dilated
    3x3 window INCLUDING the center (|c-c| = 0 never changes the max).
  - Vertical (partition) shifts via SBUF->SBUF DMA on the SP ring (x loads own
    the Activation ring); ALL vertical halo rows (same-chunk band edges and
    cross-chunk edges alike) are tiny SWDGE casting loads straight from DRAM,
    so a stencil unit of any block granularity depends only on its own xb copy.
    Horizontal shifts via free-dim slices of zero-padded persistent tiles.
    Stencil in bf16 on DVE (2x mode), 10 tensor_tensor ops per chunk: the xb
    copy lands in the middle third of a [nm | xb | nx] tile so u1|u2 fuse into
    ONE 8192-wide subtract (z[FW:3FW] - z[0:2FW] = [xb-nm | nx-xb]).
  - All DVE-only scratch is persistent (Vector-queue program order resolves
    WAW/WAR; no per-chunk pool-credit semaphores, pads memset once).
  - BCE logs + relu(x-.5) + sign(x-.5) on ScalarE: ln(x + FLT_MIN)
    reproduces torch's -100 clamp for uniform inputs (only x == 0 clamps).
    thred = R + 0.25*s + 0.25 with R = relu(x-.5), s = sign(x-.5).
  - Product-sums via TensorE diagonal matmuls accumulated in PSUM:
    a-stream rhs pieces [R_j | s_j | 1] (FD=257, the ones column yields
    sum(U) for free), b-stream [lp_j | l1p_j]; 4 round-robin accumulators
    per stream; sum(l1p) rides the ACT accum_out. Host assembles the scalar.
  - Pipeline: stencil(c-1) emitted before load(c)/field(c-1) so shift DMAs and
    SWDGE halo loads are never queued behind ACT passes or ytb casts; chunk 0
    runs as two per-image halves (block-granular loads split across both HWDGE
    rings) to shorten the cold-clock startup chain, and the LAST chunk runs as
    a half plus two single blocks so only one block's a-matmuls trail the
    final DVE op.  u is double-buffered (a-matmul WAR slack); ytb
    single-buffered (its SWDGE loads only feed lag-tolerant PSUM
    accumulation).  Aggregate DMA demand sits at ~90% of the two HWDGE rings
    + SWDGE capacity, so load/shift ring assignments are load-bearing.
"""

from contextlib import ExitStack

import numpy as np

import concourse.bacc as bacc
import concourse.tile as tile
from concourse import mybir
from concourse.bass_utils import run_bass_kernel_spmd

F32 = mybir.dt.float32
BF16 = mybir.dt.bfloat16
OP = mybir.AluOpType
AT = mybir.ActivationFunctionType

B, H, W = 16, 1024, 1024
NCORES = 8
IPC = B // NCORES          # images per core = 2
P = 128
NB = 2                     # bands per image per chunk tile
NBLK = IPC * NB            # 4 column blocks per chunk tile
NCHUNK = H // (P * NB)     # 4 chunk iterations
FW = NBLK * W              # 4096
BW = W + 4                 # padded block width
DIL = 2
TINY = 1.18e-38            # min normal fp32; ln(x+TINY) == ln(x) for x >= 2^-24

NACC = 4                   # round-robin PSUM accumulators per stream
RSTR = 260                 # rhs piece stride (els) in the [R|s|1] tile (8B-aligned)
AW = 257                   # a-stream rhs width: [R(128) | s(128) | ones(1)]
N_OUT = NACC * AW + NACC * 256 + NCHUNK


def _kernel_body(ctx, tc, yp, yt, out):
    nc = tc.nc

    xpool = ctx.enter_context(tc.tile_pool(name="xpool", bufs=2))
    xbpool = ctx.enter_context(tc.tile_pool(name="xbpool", bufs=2))
    ytpool = ctx.enter_context(tc.tile_pool(name="ytpool", bufs=1))
    fpool = ctx.enter_context(tc.tile_pool(name="fpool", bufs=2))    # lpl1p / rs1
    shpool = ctx.enter_context(tc.tile_pool(name="shpool", bufs=2))  # xu/xd
    upool = ctx.enter_context(tc.tile_pool(name="upool", bufs=2))
    single = ctx.enter_context(tc.tile_pool(name="single", bufs=1))
    psum = ctx.enter_context(tc.tile_pool(name="psum", bufs=1, space="PSUM"))

    l1pacc = single.tile([P, NCHUNK], F32)
    psum_a = [psum.tile([P, AW], F32, name=f"psum_a{k}") for k in range(NACC)]
    psum_b = [psum.tile([P, 256], F32, name=f"psum_b{k}") for k in range(NACC)]

    bias_tiny = single.tile([P, 1], F32)
    nc.gpsimd.memset(bias_tiny, TINY)
    bias_one = single.tile([P, 1], F32)
    nc.gpsimd.memset(bias_one, 1.0)
    bias_neghalf = single.tile([P, 1], F32)
    nc.gpsimd.memset(bias_neghalf, -0.5)

    zrow = single.tile([DIL, W], BF16)
    nc.gpsimd.memset(zrow, 0.0)

    # Persistent stencil scratch: all DVE-written, DVE-read tiles are allocated
    # ONCE and rewritten every chunk.  Cross-chunk WAW/WAR hazards are resolved
    # by Vector-queue program order, so the tile framework emits no per-chunk
    # pool-credit semaphores for them, and the vmax/vmin pad columns are
    # memset once instead of per chunk.
    vmax = single.tile([P, NBLK * BW], BF16, name="vmax")
    vmin = single.tile([P, NBLK * BW], BF16, name="vmin")
    for v in (vmax, vmin):
        for q in range(NBLK):
            nc.gpsimd.memset(v[:, q * BW:q * BW + 2], 0.0)
            nc.gpsimd.memset(v[:, q * BW + BW - 2:(q + 1) * BW], 0.0)
    vmax3 = vmax.rearrange("p (q w) -> p q w", q=NBLK)
    vmin3 = vmin.rearrange("p (q w) -> p q w", q=NBLK)
    sg12 = single.tile([P, 2 * FW], BF16, name="sg12")  # [va|vb] -> [nxa|nma] -> ud
    sg1 = sg12[:, 0:FW]
    sg2 = sg12[:, FW:2 * FW]

    xb_tiles = {}
    rs_tiles = {}

    n_pieces = FW // P  # 32 lhsT pieces per chunk per stream

    def chunk_src(t, c, img):
        """[NB*P, W] DRAM rows of chunk c, image img -> [P, band, w] 3D AP."""
        return t[img, c * NB * P:(c + 1) * NB * P, :].rearrange(
            "(s p) w -> p s w", p=P)

    x_tiles = {}
    yt_tiles = {}

    def load_chunk(c):
        """x/ytb loads + the ACT xb copy — issued one iteration ahead of the
        field passes so the vertical-shift DMAs (and the whole DVE chain)
        never wait behind a chunk's ln/relu/sign ACT queue."""
        x = xpool.tile([P, FW], F32, name=f"x_{c}", tag="x")
        # z = [nm | xb | nx]: xb contiguous with the DVE-written nm/nx thirds
        # so u1/u2 later fuse into ONE subtract (ud = z[FW:3FW] - z[0:2FW])
        z = xbpool.tile([P, 3 * FW], BF16, name=f"z_{c}", tag="z")
        xb = z[:, FW:2 * FW]
        ytb = ytpool.tile([P, FW], BF16, name=f"ytb_{c}", tag="ytb")
        if c == 0:
            # finest-grained startup: block loads alternate rings, copies
            # run per image so the first shift DMA starts ~15 us earlier
            qs = [nc.scalar, nc.sync, nc.scalar, nc.sync]
            for img in range(IPC):
                for b in range(NB):
                    h0 = (img * NB + b) * W
                    src = yp[img, (c * NB + b) * P:(c * NB + b + 1) * P, :]
                    qs[img * NB + b].dma_start(out=x[:, h0:h0 + W], in_=src)
            for q in range(NBLK):
                h0 = q * W
                nc.scalar.copy(out=xb[:, h0:h0 + W], in_=x[:, h0:h0 + W])
        else:
            for img in range(IPC):
                h0 = img * NB * W
                o3x = x[:, h0:h0 + NB * W].rearrange("p (s w) -> p s w", s=NB)
                # chunk 1 splits across rings: the Activation ring is still
                # draining chunk 0's blocks + xd shifts at that point
                xq = nc.sync if (c == 1 and img == 1) else nc.scalar
                xq.dma_start(out=o3x, in_=chunk_src(yp, c, img))
            nc.scalar.copy(out=xb, in_=x)
        xb_tiles[c] = z
        x_tiles[c] = x
        yt_tiles[c] = ytb

    def field_chunk(c):
        x = x_tiles[c]
        ytb = yt_tiles[c]
        # ytb cast-loads are emitted here, AFTER the stencil's tiny DRAM halo
        # loads on the same SWDGE queue: the halos gate the DVE chain, the
        # 2 MB ytb reads only feed lag-tolerant PSUM accumulation
        for img in range(IPC):
            h0 = img * NB * W
            o3y = ytb[:, h0:h0 + NB * W].rearrange("p (s w) -> p s w", s=NB)
            nc.gpsimd.dma_start(out=o3y, in_=chunk_src(yt, c, img))

        # [lp|l1p] interleaved at 128 cols: piece j occupies cols [256j, 256j+256)
        lpl1p = fpool.tile([P, 2 * FW], BF16, name=f"lpl1p_{c}", tag="lpl1p", bufs=1)
        lp4 = lpl1p.rearrange("p (j t w) -> p j t w", t=2, w=P)
        nc.scalar.activation(lp4[:, :, 0, :], x, AT.Ln, bias=bias_tiny, scale=1.0)
        nc.scalar.activation(
            lp4[:, :, 1, :], x, AT.Ln, bias=bias_one, scale=-1.0,
            accum_out=l1pacc[:, c:c + 1],
        )

        # [R|s|1] pieces with stride RSTR; R, s on ACT; ones via memset
        rs1 = fpool.tile([P, n_pieces * RSTR], BF16, name=f"rs1_{c}", tag="rs1",
                         bufs=1)
        rs4 = rs1.rearrange("p (j w) -> p j w", j=n_pieces)
        nc.scalar.activation(rs4[:, :, 0:P], x, AT.Relu, bias=bias_neghalf, scale=1.0)
        nc.scalar.activation(rs4[:, :, P:2 * P], x, AT.Sign, bias=bias_neghalf, scale=1.0)
        nc.gpsimd.memset(rs4[:, :, 2 * P:2 * P + 1], 1.0)
        rs_tiles[c] = rs1

        # BCE product-sums: psum_b[m, :] += sum_k ytb[k, 128j+m] * [lp|l1p](j)[k, :]
        for j in range(n_pieces):
            nc.tensor.matmul(
                psum_b[j % NACC],
                ytb[:, j * P:(j + 1) * P],
                lpl1p[:, j * 256:(j + 1) * 256],
                start=(c == 0 and j < NACC),
                stop=(c == NCHUNK - 1 and j >= n_pieces - NACC),
            )

    def stencil_chunk(c, q0=0, nq=NBLK):
        """Stencil over blocks [q0, q0+nq) of chunk c (q0 on an image
        boundary).  Cross-chunk vertical halos are cast-loaded straight from
        DRAM so no chunk waits on its neighbour's xb copy; chunk 0 runs as
        two per-image halves to shorten the load->copy->shift startup chain."""
        z = xb_tiles[c]
        xbc = z[:, FW:2 * FW]
        znm = z[:, 0:FW]
        znx = z[:, 2 * FW:3 * FW]
        imgs = range(q0 // NB, (q0 + nq) // NB)
        cs = slice(q0 * W, (q0 + nq) * W)

        # vertical +-2 partition shifts, both on the otherwise-idle SP ring
        # (the Activation ring is saturated by the x loads + ACT passes)
        xu = shpool.tile([P, FW], BF16, name=f"xu_{c}_{q0}", tag="xu")
        xd = shpool.tile([P, FW], BF16, name=f"xd_{c}_{q0}", tag="xd")
        nc.sync.dma_start(out=xu[0:P - DIL, cs], in_=xbc[DIL:P, cs])
        xdq = nc.scalar if c == 0 else nc.sync
        xdq.dma_start(out=xd[DIL:P, cs], in_=xbc[0:P - DIL, cs])

        # ALL vertical halo rows are tiny casting loads straight from DRAM
        # (SWDGE): no stencil unit ever waits on another block's xb copy, so
        # the unit size can be anything down to a single [128, W] block.
        # Block q = (img q//2, band q%2); band b's rows are c*256+128b+p.
        def halo(t, r0, qs_, src_row):
            """rows [r0,r0+2) of blocks qs_ <- DRAM rows src_row(q)."""
            if len(qs_) == 2:
                o = t[r0:r0 + DIL, qs_[0] * W:(qs_[0] + 3) * W].rearrange(
                    "p (i w) -> p i w", w=W)[:, ::2, :]
                i0, i1 = qs_[0] // 2, qs_[1] // 2
                r = src_row(qs_[0])
                src = yp[i0:i1 + 1, r:r + DIL, :].rearrange("i p w -> p i w")
            else:
                q = qs_[0]
                o = t[r0:r0 + DIL, q * W:(q + 1) * W]
                r = src_row(q)
                src = yp[q // 2, r:r + DIL, :]
            nc.gpsimd.dma_start(out=o, in_=src)

        evq = [q for q in range(q0, q0 + nq) if q % 2 == 0]
        odq = [q for q in range(q0, q0 + nq) if q % 2 == 1]
        base = c * NB * P
        # xu bottom halos: band0 <- same-chunk band1 top rows; band1 <- next
        # chunk band0 top rows (zero rows at the image bottom)
        if evq:
            halo(xu, P - DIL, evq, lambda q: base + P)
        if odq:
            if c + 1 < NCHUNK:
                halo(xu, P - DIL, odq, lambda q: base + NB * P)
            else:
                for q in odq:
                    nc.sync.dma_start(
                        out=xu[P - DIL:P, q * W:(q + 1) * W], in_=zrow)
        # xd top halos: band1 <- same-chunk band0 bottom rows; band0 <- prev
        # chunk band1 bottom rows (zero rows at the image top)
        if odq:
            halo(xd, 0, odq, lambda q: base + P - DIL)
        if evq:
            if c > 0:
                halo(xd, 0, evq, lambda q: base - DIL)
            else:
                for q in evq:
                    xdq.dma_start(out=xd[0:DIL, q * W:(q + 1) * W], in_=zrow)

        def b3(t):
            return t[:, cs].rearrange("p (q w) -> p q w", q=nq)

        vx = vmax3[:, q0:q0 + nq]
        vn = vmin3[:, q0:q0 + nq]

        # vertical 3-max / 3-min into the persistent zero-padded tiles
        nc.vector.tensor_tensor(out=sg1[:, cs], in0=xu[:, cs], in1=xd[:, cs],
                                op=OP.max)
        nc.vector.tensor_tensor(
            out=vx[:, :, 2:2 + W], in0=b3(sg1), in1=b3(xbc), op=OP.max)
        nc.vector.tensor_tensor(out=sg2[:, cs], in0=xu[:, cs], in1=xd[:, cs],
                                op=OP.min)
        nc.vector.tensor_tensor(
            out=vn[:, :, 2:2 + W], in0=b3(sg2), in1=b3(xbc), op=OP.min)

        # horizontal dilated 3-max / 3-min
        nc.vector.tensor_tensor(
            out=b3(sg1), in0=vx[:, :, 0:W], in1=vx[:, :, 4:4 + W], op=OP.max)
        nc.vector.tensor_tensor(
            out=b3(znx), in0=b3(sg1), in1=vx[:, :, 2:2 + W], op=OP.max)
        nc.vector.tensor_tensor(
            out=b3(sg2), in0=vn[:, :, 0:W], in1=vn[:, :, 4:4 + W], op=OP.min)
        nc.vector.tensor_tensor(
            out=b3(znm), in0=b3(sg2), in1=vn[:, :, 2:2 + W], op=OP.min)

        # ud = z[FW:3FW] - z[0:2FW] = [xb - nm | nx - xb] in ONE subtract;
        # unfolded u = max of the two halves
        du = sg12.rearrange("p (t w) -> p t w", t=2)[:, :, cs]
        i0 = z[:, FW:3 * FW].rearrange("p (t w) -> p t w", t=2)[:, :, cs]
        i1 = z[:, 0:2 * FW].rearrange("p (t w) -> p t w", t=2)[:, :, cs]
        nc.vector.tensor_tensor(out=du, in0=i0, in1=i1, op=OP.subtract)
        u = upool.tile([P, FW], BF16, name=f"u_{c}_{q0}", tag="u")
        nc.vector.tensor_tensor(out=u[:, cs], in0=sg1[:, cs], in1=sg2[:, cs],
                                op=OP.max)
        return u

    def amm_chunk(c, u, q0=0, nq=NBLK):
        # psum_a[m, :] += sum_k u[k, 128j+m] * [R|s|1](j)[k, :]
        # (emitted after field_chunk so the rs1 writes precede these reads)
        rsc = rs_tiles[c]
        for j in range(q0 * (W // P), (q0 + nq) * (W // P)):
            nc.tensor.matmul(
                psum_a[j % NACC],
                u[:, j * P:(j + 1) * P],
                rsc[:, j * RSTR:j * RSTR + AW],
                start=(c == 0 and j < NACC),
                stop=(c == NCHUNK - 1 and j >= n_pieces - NACC),
            )

    def drain_b():
        # psum_b completes with the last load_chunk; copy out early so the
        # endgame only waits on the a-stream (copies on ScalarE: close to PSUM)
        for k in range(NACC):
            res = single.tile([P, 256], F32, name=f"resb_{k}", tag="resb", bufs=2)
            nc.scalar.copy(out=res, in_=psum_b[k])
            nc.sync.dma_start(
                out=out[:, NACC * AW + k * 256:NACC * AW + (k + 1) * 256], in_=res)
        nc.sync.dma_start(out=out[:, NACC * (AW + 256):N_OUT], in_=l1pacc)

    # software pipeline: loads lead the field/stencil passes by one chunk;
    # stencil is emitted first so its shift DMAs aren't queued behind the
    # chunk's ln/relu/sign ACT work; chunk 0 runs as two per-image halves
    # to shorten the startup latency chain
    for c in range(NCHUNK + 1):
        if c >= 1:
            cc = c - 1
            if cc == 0:
                # first chunk: two single blocks then a half - the first DVE
                # op then needs only one 512 KB block load + one [128, W]
                # copy + one 0.25 MB shift, without paying the per-op bubble
                # tax of single-block units across the whole chunk
                us = [(stencil_chunk(0, 0, 1), 0, 1),
                      (stencil_chunk(0, 1, 1), 1, 1),
                      (stencil_chunk(0, NB, NB), NB, NB)]
            elif cc == NCHUNK - 1:
                # last chunk: finer tail units so most a-matmuls overlap the
                # remaining DVE work and only one block's worth trails it
                us = [(stencil_chunk(cc, 0, NB), 0, NB),
                      (stencil_chunk(cc, NB, 1), NB, 1),
                      (stencil_chunk(cc, NB + 1, 1), NB + 1, 1)]
            else:
                us = [(stencil_chunk(cc), 0, NBLK)]
        if c < NCHUNK:
            load_chunk(c)
        if c >= 1:
            field_chunk(cc)
            if c == NCHUNK:
                drain_b()
            for u, q0, nq in us:
                amm_chunk(cc, u, q0, nq)

    for k in range(NACC):
        res = single.tile([P, AW], F32, name=f"resa_{k}", tag="resa", bufs=3)
        nc.scalar.copy(out=res, in_=psum_a[k])
        nc.sync.dma_start(out=out[:, k * AW:(k + 1) * AW], in_=res)


_CACHED = {}


def _build():
    if "nc" in _CACHED:
        return _CACHED["nc"]
    nc = bacc.Bacc(
        "TRN2",
        target_bir_lowering=False,
        debug=False,
        num_devices=NCORES,
    )
    yp = nc.dram_tensor("y_pred", [IPC, H, W], F32, kind="ExternalInput").ap()
    yt = nc.dram_tensor("y_true", [IPC, H, W], F32, kind="ExternalInput").ap()
    out = nc.dram_tensor("out", [P, N_OUT], F32, kind="ExternalOutput").ap()
    with tile.TileContext(nc) as tc:
        with ExitStack() as ctx:
            _kernel_body(ctx, tc, yp, yt, out)
    nc.compile()
    _CACHED["nc"] = nc
    return nc


def _host_reduce(outs):
    """Assemble the scalar loss from the 8 per-core [P, N_OUT] partial tensors."""
    total = np.float64(0.0)
    idx = np.arange(P)
    for o in outs:
        o = np.asarray(o, dtype=np.float64)
        a = o[:, 0:NACC * AW].reshape(P, NACC, AW).sum(axis=1)
        bq = o[:, NACC * AW:NACC * (AW + 256)].reshape(P, NACC, 256).sum(axis=1)
        l1 = o[:, NACC * (AW + 256):NACC * (AW + 256) + NCHUNK]
        sum_ur = a[idx, idx].sum()          # sum U * relu(x-.5)
        sum_us = a[idx, 128 + idx].sum()    # sum U * sign(x-.5)
        sum_u = a[:, 256].sum()             # sum U
        sum_ylp = bq[idx, idx].sum()        # sum yt * ln(x)
        sum_yl1p = bq[idx, 128 + idx].sum() # sum yt * ln(1-x)
        sum_l1p = l1.sum()                  # sum ln(1-x)
        # thred = R + 0.25*s + 0.25
        total += (sum_ur + 0.25 * sum_us + 0.25 * sum_u) \
            - sum_ylp - sum_l1p + sum_yl1p
    return np.float32(total / (B * H * W))


def kernel(y_true, y_pred):
    y_true = np.ascontiguousarray(np.asarray(y_true, dtype=np.float32)).reshape(B, H, W)
    y_pred = np.ascontiguousarray(np.asarray(y_pred, dtype=np.float32)).reshape(B, H, W)

    nc = _build()
    in_maps = []
    for r in range(NCORES):
        in_maps.append({
            "y_pred": np.ascontiguousarray(y_pred[r * IPC:(r + 1) * IPC]),
            "y_true": np.ascontiguousarray(y_true[r * IPC:(r + 1) * IPC]),
        })
    res = run_bass_kernel_spmd(nc, in_maps, core_ids=list(range(NCORES)))
    outs = [res.results[r]["out"] for r in range(NCORES)]
    return _host_reduce(outs)



# revision 35
# speedup vs baseline: 1.1839x; 1.1839x over previous
"""Trainium2 Bass kernel for nn_ConsistencyLoss (BCE + dilated-stencil consistency loss).

loss = mean( unfolded_weights * thred + bce )
  bce      = -(y_true*max(log(y_pred),-100) + (1-y_true)*max(log1p(-y_pred),-100))
  unfolded = max over 8 dilated (DIL=2) neighbors nb of |y_pred - nb|, zero-padded
  thred    = y_pred * (y_pred >= 0.5)

Strategy (8 NeuronCores, data-parallel over batch, 2 images/core):
  - Chunk tiles [128, 4096] = 2 bands x 2 images, blocks [i0b0|i0b1|i1b0|i1b1].
  - unfolded = max(c - nmin, nmax - c); nmax/nmin separable over the dilated
    3x3 window INCLUDING the center (|c-c| = 0 never changes the max).
  - Vertical (partition) shifts via SBUF->SBUF DMA on the SP ring (x loads own
    the Activation ring); ALL vertical halo rows (same-chunk band edges and
    cross-chunk edges alike) are tiny SWDGE casting loads straight from DRAM,
    so a stencil unit of any block granularity depends only on its own xb copy.
    Horizontal shifts via free-dim slices of zero-padded persistent tiles.
    Stencil in bf16 on DVE (2x mode), 10 tensor_tensor ops per chunk: the xb
    copy lands in the middle third of a [nm | xb | nx] tile so u1|u2 fuse into
    ONE 8192-wide subtract (z[FW:3FW] - z[0:2FW] = [xb-nm | nx-xb]).
  - All DVE-only scratch is persistent (Vector-queue program order resolves
    WAW/WAR; no per-chunk pool-credit semaphores, pads memset once).
  - BCE logs + relu(x-.5) + sign(x-.5) on ScalarE: ln(x + FLT_MIN)
    reproduces torch's -100 clamp for uniform inputs (only x == 0 clamps).
    thred = R + 0.25*s + 0.25 with R = relu(x-.5), s = sign(x-.5).
  - Product-sums via TensorE diagonal matmuls accumulated in PSUM:
    a-stream rhs pieces [R_j | s_j | 1] (FD=257, the ones column yields
    sum(U) for free), b-stream [lp_j | l1p_j]; 4 round-robin accumulators
    per stream; sum(l1p) rides the ACT accum_out. Host assembles the scalar.
  - Pipeline: stencil(c-1) emitted before load(c)/field(c-1) so shift DMAs and
    SWDGE halo loads are never queued behind ACT passes or ytb casts; chunk 0
    runs as two per-image halves (block-granular loads split across both HWDGE
    rings) to shorten the cold-clock startup chain, and the LAST chunk runs as
    a half plus two single blocks so only one block's a-matmuls trail the
    final DVE op.  u is double-buffered (a-matmul WAR slack); ytb
    single-buffered (its SWDGE loads only feed lag-tolerant PSUM
    accumulation).  Aggregate DMA demand sits at ~90% of the two HWDGE rings
    + SWDGE capacity, so load/shift ring assignments are load-bearing.
"""

from contextlib import ExitStack

import numpy as np

import concourse.bacc as bacc
import concourse.tile as tile
from concourse import mybir
from concourse.bass_utils import run_bass_kernel_spmd

F32 = mybir.dt.float32
BF16 = mybir.dt.bfloat16
OP = mybir.AluOpType
AT = mybir.ActivationFunctionType

B, H, W = 16, 1024, 1024
NCORES = 8
IPC = B // NCORES          # images per core = 2
P = 128
NB = 2                     # bands per image per chunk tile
NBLK = IPC * NB            # 4 column blocks per chunk tile
NCHUNK = H // (P * NB)     # 4 chunk iterations
FW = NBLK * W              # 4096
BW = W + 4                 # padded block width
DIL = 2
TINY = 1.18e-38            # min normal fp32; ln(x+TINY) == ln(x) for x >= 2^-24

NACC = 4                   # round-robin PSUM accumulators per stream
RSTR = 260                 # rhs piece stride (els) in the [R|s|1] tile (8B-aligned)
AW = 257                   # a-stream rhs width: [R(128) | s(128) | ones(1)]
N_OUT = NACC * AW + NACC * 256 + NCHUNK


def _kernel_body(ctx, tc, yp, yt, out):
    nc = tc.nc

    xpool = ctx.enter_context(tc.tile_pool(name="xpool", bufs=2))
    xbpool = ctx.enter_context(tc.tile_pool(name="xbpool", bufs=2))
    ytpool = ctx.enter_context(tc.tile_pool(name="ytpool", bufs=1))
    fpool = ctx.enter_context(tc.tile_pool(name="fpool", bufs=2))    # lpl1p / rs1
    shpool = ctx.enter_context(tc.tile_pool(name="shpool", bufs=2))  # xu/xd
    upool = ctx.enter_context(tc.tile_pool(name="upool", bufs=2))
    single = ctx.enter_context(tc.tile_pool(name="single", bufs=1))
    psum = ctx.enter_context(tc.tile_pool(name="psum", bufs=1, space="PSUM"))

    l1pacc = single.tile([P, NCHUNK], F32)
    psum_a = [psum.tile([P, AW], F32, name=f"psum_a{k}") for k in range(NACC)]
    psum_b = [psum.tile([P, 256], F32, name=f"psum_b{k}") for k in range(NACC)]

    bias_tiny = single.tile([P, 1], F32)
    nc.gpsimd.memset(bias_tiny, TINY)
    bias_one = single.tile([P, 1], F32)
    nc.gpsimd.memset(bias_one, 1.0)
    bias_neghalf = single.tile([P, 1], F32)
    nc.gpsimd.memset(bias_neghalf, -0.5)

    zrow = single.tile([DIL, W], BF16)
    nc.gpsimd.memset(zrow, 0.0)

    # Persistent stencil scratch: all DVE-written, DVE-read tiles are allocated
    # ONCE and rewritten every chunk.  Cross-chunk WAW/WAR hazards are resolved
    # by Vector-queue program order, so the tile framework emits no per-chunk
    # pool-credit semaphores for them, and the vmax/vmin pad columns are
    # memset once instead of per chunk.
    vmax = single.tile([P, NBLK * BW], BF16, name="vmax")
    vmin = single.tile([P, NBLK * BW], BF16, name="vmin")
    for v in (vmax, vmin):
        for q in range(NBLK):
            nc.gpsimd.memset(v[:, q * BW:q * BW + 2], 0.0)
            nc.gpsimd.memset(v[:, q * BW + BW - 2:(q + 1) * BW], 0.0)
    vmax3 = vmax.rearrange("p (q w) -> p q w", q=NBLK)
    vmin3 = vmin.rearrange("p (q w) -> p q w", q=NBLK)
    sg12 = single.tile([P, 2 * FW], BF16, name="sg12")  # [va|vb] -> [nxa|nma] -> ud
    sg1 = sg12[:, 0:FW]
    sg2 = sg12[:, FW:2 * FW]

    xb_tiles = {}
    rs_tiles = {}

    n_pieces = FW // P  # 32 lhsT pieces per chunk per stream

    def chunk_src(t, c, img):
        """[NB*P, W] DRAM rows of chunk c, image img -> [P, band, w] 3D AP."""
        return t[img, c * NB * P:(c + 1) * NB * P, :].rearrange(
            "(s p) w -> p s w", p=P)

    x_tiles = {}
    yt_tiles = {}

    def load_chunk(c):
        """x/ytb loads + the ACT xb copy — issued one iteration ahead of the
        field passes so the vertical-shift DMAs (and the whole DVE chain)
        never wait behind a chunk's ln/relu/sign ACT queue."""
        x = xpool.tile([P, FW], F32, name=f"x_{c}", tag="x")
        # z = [nm | xb | nx]: xb contiguous with the DVE-written nm/nx thirds
        # so u1/u2 later fuse into ONE subtract (ud = z[FW:3FW] - z[0:2FW])
        z = xbpool.tile([P, 3 * FW], BF16, name=f"z_{c}", tag="z")
        xb = z[:, FW:2 * FW]
        ytb = ytpool.tile([P, FW], BF16, name=f"ytb_{c}", tag="ytb")
        if c == 0:
            # finest-grained startup: block loads alternate rings, copies
            # run per image so the first shift DMA starts ~15 us earlier
            qs = [nc.scalar, nc.sync, nc.scalar, nc.scalar]
            for img in range(IPC):
                for b in range(NB):
                    h0 = (img * NB + b) * W
                    src = yp[img, (c * NB + b) * P:(c * NB + b + 1) * P, :]
                    qs[img * NB + b].dma_start(out=x[:, h0:h0 + W], in_=src)
            for q in range(NBLK):
                h0 = q * W
                nc.scalar.copy(out=xb[:, h0:h0 + W], in_=x[:, h0:h0 + W])
        else:
            for img in range(IPC):
                h0 = img * NB * W
                o3x = x[:, h0:h0 + NB * W].rearrange("p (s w) -> p s w", s=NB)
                # chunk 1 splits across rings: the Activation ring is still
                # draining chunk 0's blocks + xd shifts at that point
                xq = nc.sync if (c == 1 and img == 1) else nc.scalar
                xq.dma_start(out=o3x, in_=chunk_src(yp, c, img))
            nc.scalar.copy(out=xb, in_=x)
        xb_tiles[c] = z
        x_tiles[c] = x
        yt_tiles[c] = ytb

    def field_chunk(c):
        x = x_tiles[c]
        ytb = yt_tiles[c]
        # ytb cast-loads are emitted here, AFTER the stencil's tiny DRAM halo
        # loads on the same SWDGE queue: the halos gate the DVE chain, the
        # 2 MB ytb reads only feed lag-tolerant PSUM accumulation
        for img in range(IPC):
            h0 = img * NB * W
            o3y = ytb[:, h0:h0 + NB * W].rearrange("p (s w) -> p s w", s=NB)
            nc.gpsimd.dma_start(out=o3y, in_=chunk_src(yt, c, img))

        # [lp|l1p] interleaved at 128 cols: piece j occupies cols [256j, 256j+256)
        lpl1p = fpool.tile([P, 2 * FW], BF16, name=f"lpl1p_{c}", tag="lpl1p", bufs=1)
        lp4 = lpl1p.rearrange("p (j t w) -> p j t w", t=2, w=P)
        nc.scalar.activation(lp4[:, :, 0, :], x, AT.Ln, bias=bias_tiny, scale=1.0)
        nc.scalar.activation(
            lp4[:, :, 1, :], x, AT.Ln, bias=bias_one, scale=-1.0,
            accum_out=l1pacc[:, c:c + 1],
        )

        # [R|s|1] pieces with stride RSTR; R, s on ACT; ones via memset
        rs1 = fpool.tile([P, n_pieces * RSTR], BF16, name=f"rs1_{c}", tag="rs1",
                         bufs=1)
        rs4 = rs1.rearrange("p (j w) -> p j w", j=n_pieces)
        nc.scalar.activation(rs4[:, :, 0:P], x, AT.Relu, bias=bias_neghalf, scale=1.0)
        nc.scalar.activation(rs4[:, :, P:2 * P], x, AT.Sign, bias=bias_neghalf, scale=1.0)
        nc.gpsimd.memset(rs4[:, :, 2 * P:2 * P + 1], 1.0)
        rs_tiles[c] = rs1

        # BCE product-sums: psum_b[m, :] += sum_k ytb[k, 128j+m] * [lp|l1p](j)[k, :]
        for j in range(n_pieces):
            nc.tensor.matmul(
                psum_b[j % NACC],
                ytb[:, j * P:(j + 1) * P],
                lpl1p[:, j * 256:(j + 1) * 256],
                start=(c == 0 and j < NACC),
                stop=(c == NCHUNK - 1 and j >= n_pieces - NACC),
            )

    def stencil_chunk(c, q0=0, nq=NBLK):
        """Stencil over blocks [q0, q0+nq) of chunk c (q0 on an image
        boundary).  Cross-chunk vertical halos are cast-loaded straight from
        DRAM so no chunk waits on its neighbour's xb copy; chunk 0 runs as
        two per-image halves to shorten the load->copy->shift startup chain."""
        z = xb_tiles[c]
        xbc = z[:, FW:2 * FW]
        znm = z[:, 0:FW]
        znx = z[:, 2 * FW:3 * FW]
        imgs = range(q0 // NB, (q0 + nq) // NB)
        cs = slice(q0 * W, (q0 + nq) * W)

        # vertical +-2 partition shifts, both on the otherwise-idle SP ring
        # (the Activation ring is saturated by the x loads + ACT passes)
        xu = shpool.tile([P, FW], BF16, name=f"xu_{c}_{q0}", tag="xu")
        xd = shpool.tile([P, FW], BF16, name=f"xd_{c}_{q0}", tag="xd")
        nc.sync.dma_start(out=xu[0:P - DIL, cs], in_=xbc[DIL:P, cs])
        xdq = nc.scalar if c == 0 else nc.sync
        xdq.dma_start(out=xd[DIL:P, cs], in_=xbc[0:P - DIL, cs])

        # ALL vertical halo rows are tiny casting loads straight from DRAM
        # (SWDGE): no stencil unit ever waits on another block's xb copy, so
        # the unit size can be anything down to a single [128, W] block.
        # Block q = (img q//2, band q%2); band b's rows are c*256+128b+p.
        def halo(t, r0, qs_, src_row):
            """rows [r0,r0+2) of blocks qs_ <- DRAM rows src_row(q)."""
            if len(qs_) == 2:
                o = t[r0:r0 + DIL, qs_[0] * W:(qs_[0] + 3) * W].rearrange(
                    "p (i w) -> p i w", w=W)[:, ::2, :]
                i0, i1 = qs_[0] // 2, qs_[1] // 2
                r = src_row(qs_[0])
                src = yp[i0:i1 + 1, r:r + DIL, :].rearrange("i p w -> p i w")
            else:
                q = qs_[0]
                o = t[r0:r0 + DIL, q * W:(q + 1) * W]
                r = src_row(q)
                src = yp[q // 2, r:r + DIL, :]
            nc.gpsimd.dma_start(out=o, in_=src)

        evq = [q for q in range(q0, q0 + nq) if q % 2 == 0]
        odq = [q for q in range(q0, q0 + nq) if q % 2 == 1]
        base = c * NB * P
        # xu bottom halos: band0 <- same-chunk band1 top rows; band1 <- next
        # chunk band0 top rows (zero rows at the image bottom)
        if evq:
            halo(xu, P - DIL, evq, lambda q: base + P)
        if odq:
            if c + 1 < NCHUNK:
                halo(xu, P - DIL, odq, lambda q: base + NB * P)
            else:
                for q in odq:
                    nc.sync.dma_start(
                        out=xu[P - DIL:P, q * W:(q + 1) * W], in_=zrow)
        # xd top halos: band1 <- same-chunk band0 bottom rows; band0 <- prev
        # chunk band1 bottom rows (zero rows at the image top)
        if odq:
            halo(xd, 0, odq, lambda q: base + P - DIL)
        if evq:
            if c > 0:
                halo(xd, 0, evq, lambda q: base - DIL)
            else:
                for q in evq:
                    xdq.dma_start(out=xd[0:DIL, q * W:(q + 1) * W], in_=zrow)

        def b3(t):
            return t[:, cs].rearrange("p (q w) -> p q w", q=nq)

        vx = vmax3[:, q0:q0 + nq]
        vn = vmin3[:, q0:q0 + nq]

        # vertical 3-max / 3-min into the persistent zero-padded tiles
        nc.vector.tensor_tensor(out=sg1[:, cs], in0=xu[:, cs], in1=xd[:, cs],
                                op=OP.max)
        nc.vector.tensor_tensor(
            out=vx[:, :, 2:2 + W], in0=b3(sg1), in1=b3(xbc), op=OP.max)
        nc.vector.tensor_tensor(out=sg2[:, cs], in0=xu[:, cs], in1=xd[:, cs],
                                op=OP.min)
        nc.vector.tensor_tensor(
            out=vn[:, :, 2:2 + W], in0=b3(sg2), in1=b3(xbc), op=OP.min)

        # horizontal dilated 3-max / 3-min
        nc.vector.tensor_tensor(
            out=b3(sg1), in0=vx[:, :, 0:W], in1=vx[:, :, 4:4 + W], op=OP.max)
        nc.vector.tensor_tensor(
            out=b3(znx), in0=b3(sg1), in1=vx[:, :, 2:2 + W], op=OP.max)
        nc.vector.tensor_tensor(
            out=b3(sg2), in0=vn[:, :, 0:W], in1=vn[:, :, 4:4 + W], op=OP.min)
        nc.vector.tensor_tensor(
            out=b3(znm), in0=b3(sg2), in1=vn[:, :, 2:2 + W], op=OP.min)

        # ud = z[FW:3FW] - z[0:2FW] = [xb - nm | nx - xb] in ONE subtract;
        # unfolded u = max of the two halves
        du = sg12.rearrange("p (t w) -> p t w", t=2)[:, :, cs]
        i0 = z[:, FW:3 * FW].rearrange("p (t w) -> p t w", t=2)[:, :, cs]
        i1 = z[:, 0:2 * FW].rearrange("p (t w) -> p t w", t=2)[:, :, cs]
        nc.vector.tensor_tensor(out=du, in0=i0, in1=i1, op=OP.subtract)
        u = upool.tile([P, FW], BF16, name=f"u_{c}_{q0}", tag="u")
        nc.vector.tensor_tensor(out=u[:, cs], in0=sg1[:, cs], in1=sg2[:, cs],
                                op=OP.max)
        return u

    def amm_chunk(c, u, q0=0, nq=NBLK):
        # psum_a[m, :] += sum_k u[k, 128j+m] * [R|s|1](j)[k, :]
        # (emitted after field_chunk so the rs1 writes precede these reads)
        rsc = rs_tiles[c]
        for j in range(q0 * (W // P), (q0 + nq) * (W // P)):
            nc.tensor.matmul(
                psum_a[j % NACC],
                u[:, j * P:(j + 1) * P],
                rsc[:, j * RSTR:j * RSTR + AW],
                start=(c == 0 and j < NACC),
                stop=(c == NCHUNK - 1 and j >= n_pieces - NACC),
            )

    def drain_b():
        # psum_b completes with the last load_chunk; copy out early so the
        # endgame only waits on the a-stream (copies on ScalarE: close to PSUM)
        for k in range(NACC):
            res = single.tile([P, 256], F32, name=f"resb_{k}", tag="resb", bufs=2)
            nc.scalar.copy(out=res, in_=psum_b[k])
            nc.sync.dma_start(
                out=out[:, NACC * AW + k * 256:NACC * AW + (k + 1) * 256], in_=res)
        nc.sync.dma_start(out=out[:, NACC * (AW + 256):N_OUT], in_=l1pacc)

    # software pipeline: loads lead the field/stencil passes by one chunk;
    # stencil is emitted first so its shift DMAs aren't queued behind the
    # chunk's ln/relu/sign ACT work; chunk 0 runs as two per-image halves
    # to shorten the startup latency chain
    for c in range(NCHUNK + 1):
        if c >= 1:
            cc = c - 1
            if cc == 0:
                # first chunk: per-image halves shorten the cold-start
                # latency chain without doubling per-op bubble/sem overhead
                us = [(stencil_chunk(0, 0, NB), 0, NB),
                      (stencil_chunk(0, NB, NB), NB, NB)]
            elif cc == NCHUNK - 1:
                # last chunk: finer tail units so most a-matmuls overlap the
                # remaining DVE work and only one block's worth trails it
                us = [(stencil_chunk(cc, 0, NB), 0, NB),
                      (stencil_chunk(cc, NB, 1), NB, 1),
                      (stencil_chunk(cc, NB + 1, 1), NB + 1, 1)]
            else:
                us = [(stencil_chunk(cc), 0, NBLK)]
        if c < NCHUNK:
            load_chunk(c)
        if c >= 1:
            field_chunk(cc)
            if c == NCHUNK:
                drain_b()
            for u, q0, nq in us:
                amm_chunk(cc, u, q0, nq)

    for k in range(NACC):
        res = single.tile([P, AW], F32, name=f"resa_{k}", tag="resa", bufs=3)
        nc.scalar.copy(out=res, in_=psum_a[k])
        nc.sync.dma_start(out=out[:, k * AW:(k + 1) * AW], in_=res)


_CACHED = {}


def _build():
    if "nc" in _CACHED:
        return _CACHED["nc"]
    nc = bacc.Bacc(
        "TRN2",
        target_bir_lowering=False,
        debug=False,
        num_devices=NCORES,
    )
    yp = nc.dram_tensor("y_pred", [IPC, H, W], F32, kind="ExternalInput").ap()
    yt = nc.dram_tensor("y_true", [IPC, H, W], F32, kind="ExternalInput").ap()
    out = nc.dram_tensor("out", [P, N_OUT], F32, kind="ExternalOutput").ap()
    with tile.TileContext(nc) as tc:
        with ExitStack() as ctx:
            _kernel_body(ctx, tc, yp, yt, out)
    nc.compile()
    _CACHED["nc"] = nc
    return nc


def _host_reduce(outs):
    """Assemble the scalar loss from the 8 per-core [P, N_OUT] partial tensors."""
    total = np.float64(0.0)
    idx = np.arange(P)
    for o in outs:
        o = np.asarray(o, dtype=np.float64)
        a = o[:, 0:NACC * AW].reshape(P, NACC, AW).sum(axis=1)
        bq = o[:, NACC * AW:NACC * (AW + 256)].reshape(P, NACC, 256).sum(axis=1)
        l1 = o[:, NACC * (AW + 256):NACC * (AW + 256) + NCHUNK]
        sum_ur = a[idx, idx].sum()          # sum U * relu(x-.5)
        sum_us = a[idx, 128 + idx].sum()    # sum U * sign(x-.5)
        sum_u = a[:, 256].sum()             # sum U
        sum_ylp = bq[idx, idx].sum()        # sum yt * ln(x)
        sum_yl1p = bq[idx, 128 + idx].sum() # sum yt * ln(1-x)
        sum_l1p = l1.sum()                  # sum ln(1-x)
        # thred = R + 0.25*s + 0.25
        total += (sum_ur + 0.25 * sum_us + 0.25 * sum_u) \
            - sum_ylp - sum_l1p + sum_yl1p
    return np.float32(total / (B * H * W))


def kernel(y_true, y_pred):
    y_true = np.ascontiguousarray(np.asarray(y_true, dtype=np.float32)).reshape(B, H, W)
    y_pred = np.ascontiguousarray(np.asarray(y_pred, dtype=np.float32)).reshape(B, H, W)

    nc = _build()
    in_maps = []
    for r in range(NCORES):
        in_maps.append({
            "y_pred": np.ascontiguousarray(y_pred[r * IPC:(r + 1) * IPC]),
            "y_true": np.ascontiguousarray(y_true[r * IPC:(r + 1) * IPC]),
        })
    res = run_bass_kernel_spmd(nc, in_maps, core_ids=list(range(NCORES)))
    outs = [res.results[r]["out"] for r in range(NCORES)]
    return _host_reduce(outs)

